# revision 46
# baseline (speedup 1.0000x reference)
"""MQA attention (32 query heads, 1 KV head, ALiBi, causal) on 8 trn2 cores.

Sharding: tensor-parallel over query heads (4 heads/core). Wq rows and Wo
columns are sharded; x, Wk, Wv are replicated. Each core computes a partial
[T, E] output (its 4 heads through its Wo column-shard) in bf16; the host
sums the 8 partials in fp32.

All matmuls run in bf16 (1 cycle/row on PE regardless of width). Per core:

  proj (PE):   qT_h = (Wq_h D^-.5) @ x^T [64,T]/head, kvT = Wkv @ x^T [128,T]
  scores (PE): ST[k,q] = kTa[:,kblk]^T qTa[:,qcols]  (aug row: ones x -s_h i)
  exp (ACT):   g = exp(ST + s_h(128 tk + p))  bias per (head, k-tile)
  mask (Pool): affine_select zeroes j>i on diagonal 128-blocks
  AV (PE):     av[128q, 65] += g_blk[128k,128q]^T @ v_aug[128k,65]
               (col 64 of v_aug is ones -> softmax denominator)
  norm (DVE):  hsb[p, m, d] = av[p,m,d] * recip(av[p,m,64])
  transpose:   DMA-xbar hsb -> otn[d, qcols] (headout^T, per (h,q))
  Wo (PE):     part[t,:] = otn0[:,tblk]^T wo0 + otn1[:,tblk]^T wo1

The -s_h*i aug row cancels exactly in softmax normalization (per-query
constant), so bf16 rounding of it is harmless. Emission interleaves three
streams (proj / scores+AV / Wo) with a proportional merge so the PE stays
fed while ACT drains the exps.
"""

import numpy as np
import ml_dtypes

import concourse.bacc as bacc
import concourse.bass as bass
import concourse.mybir as mybir
import concourse.tile as tile
from concourse.bass_utils import run_bass_kernel_spmd

T = 2048
E = 2048
H = 32
D = 64
NCORES = 8
HL = H // NCORES   # 4 heads per core
ES = HL * D        # 256 per-core E shard
TQ = 512           # query tile
NTQ = T // TQ      # 4
NE = E // 128      # 16 contraction chunks
NT128 = T // 128   # 16

F32 = mybir.dt.float32
BF16 = mybir.dt.bfloat16
EXP = mybir.ActivationFunctionType.Exp

_CACHE = {}
_MM_LABELS = []

# merge weights: higher -> stream finishes earlier in the emission
W_PROJ = 2.6
W_SA = 1.0


def _merge(streams, collect=None):
    """streams: list of (steps, weight); steps = list of (cost, fn).
    Emits every fn once (or appends to `collect`), proportionally by
    weighted cumulative cost."""
    totals = [max(sum(c for c, _ in s), 1e-9) * w for s, w in streams]
    done = [0.0] * len(streams)
    idx = [0] * len(streams)
    while True:
        best, bestv = -1, None
        for i, (s, _w) in enumerate(streams):
            if idx[i] >= len(s):
                continue
            v = done[i] / totals[i]
            if best < 0 or v < bestv:
                best, bestv = i, v
        if best < 0:
            return
        step = streams[best][0][idx[best]]
        idx[best] += 1
        done[best] += step[0]
        if collect is not None:
            collect.append(step)
        else:
            step[1]()


def _build_nc():
    nc = bacc.Bacc("TRN2")
    xT = nc.dram_tensor("xT", [E, T], BF16, kind="ExternalInput")
    wqT = nc.dram_tensor("wqT", [E, ES], BF16, kind="ExternalInput")
    wkvT = nc.dram_tensor("wkvT", [E, 2 * D], BF16, kind="ExternalInput")
    woT = nc.dram_tensor("woT", [ES, E], BF16, kind="ExternalInput")
    qaug = nc.dram_tensor("qaug", [HL, T], BF16, kind="ExternalInput")
    ones = nc.dram_tensor("ones", [1, T], BF16, kind="ExternalInput")
    btbl = nc.dram_tensor("btbl", [128, HL * NT128], F32, kind="ExternalInput")
    part = nc.dram_tensor("part", [T, E], BF16, kind="ExternalOutput")

    from contextlib import ExitStack
    with tile.TileContext(nc) as tc, ExitStack() as ctx:
        _body(nc, tc, ctx, xT, wqT, wkvT, woT, qaug, ones, btbl, part)
    nc.finalize()
    return nc


def _body(nc, tc, ctx, xT, wqT, wkvT, woT, qaug, ones, btbl, part):
    const = ctx.enter_context(tc.tile_pool(name="const", bufs=1))
    xtp = ctx.enter_context(tc.tile_pool(name="xt", bufs=2))
    gp = ctx.enter_context(tc.tile_pool(name="g", bufs=1))
    sp = ctx.enter_context(tc.tile_pool(name="stage", bufs=2))
    obp = ctx.enter_context(tc.tile_pool(name="ob", bufs=2))
    ps = ctx.enter_context(tc.tile_pool(name="ps", bufs=1, space="PSUM"))

    # ---------------- resident constants (DMAs emitted in bootstrap) ----
    wq_res = const.tile([128, NE, ES], BF16)
    wkv_res = const.tile([128, NE, 2 * D], BF16)
    wo_res = [const.tile([128, E], BF16, tag=f"wo{p2}", name=f"wo{p2}")
              for p2 in range(2)]
    qTa = [const.tile([65, T], BF16, tag=f"qTa{h}", name=f"qTa{h}")
           for h in range(HL)]
    kTa = const.tile([65, T], BF16)
    # v_aug: [128 keys, k-tile, 128 slot]; col 64 = ones (denominator row)
    v_aug = const.tile([128, NT128, 128], BF16)
    btbl_t = const.tile([128, HL * NT128], F32)
    otn = [const.tile([128, T], BF16, tag=f"otn{p2}", name=f"otn{p2}")
           for p2 in range(2)]

    # live tiles, stashed at emission time by the creating step
    live = {}

    def load_consts_early():
        # kv weights first (P(0) starts with the kv group), then x chunks
        # interleaved with wq so each proj group starts as data lands
        xt0 = xtp.tile([128, NE, TQ], BF16, tag="xt", name="xt0")
        live[("xt", 0)] = xt0
        for e8 in range(2):
            sl = slice(e8 * 1024, (e8 + 1) * 1024)
            nc.sync.dma_start(
                out=wkv_res[:, 8 * e8:8 * e8 + 8, :],
                in_=wkvT[sl, :].rearrange("(e p) m -> p e m", p=128))
        for e4 in range(NE // 4):
            sl = slice(e4 * 512, (e4 + 1) * 512)
            nc.sync.dma_start(
                out=xt0[:, 4 * e4:4 * e4 + 4, :],
                in_=xT[sl, 0:TQ].rearrange("(e p) t -> p e t", p=128))
            nc.sync.dma_start(
                out=wq_res[:, 4 * e4:4 * e4 + 4, :],
                in_=wqT[sl, :].rearrange("(e p) m -> p e m", p=128))

    def load_consts_mid():
        for h in range(HL):
            nc.sync.dma_start(out=qTa[h][64:65, :], in_=qaug[h:h + 1, :])
        nc.sync.dma_start(out=btbl_t, in_=btbl[:, :])
        nc.gpsimd.memset(kTa[64:65, :], 1.0)
        nc.gpsimd.memset(v_aug[:, :, 64:65], 1.0)

    def load_consts_late():
        for p2 in range(2):
            nc.sync.dma_start(out=wo_res[p2],
                              in_=woT[p2 * 128:(p2 + 1) * 128, :])

    # ---------------- proj stream (kv group first) ----------------
    def proj_steps_q(q):
        steps = []
        if True:
            cs, ce = q * TQ, (q + 1) * TQ

            if q > 0:
                def load_x(q=q, cs=cs, ce=ce):
                    xt = xtp.tile([128, NE, TQ], BF16, tag="xt",
                                  name=f"xt{q}")
                    live[("xt", q)] = xt
                    for e8 in range(2):
                        sl = slice(e8 * 1024, (e8 + 1) * 1024)
                        nc.sync.dma_start(
                            out=xt[:, 8 * e8:8 * e8 + 8, :],
                            in_=xT[sl, cs:ce].rearrange(
                                "(e p) t -> p e t", p=128))
                    if q == 1:
                        load_consts_mid()
                    if q == 2:
                        load_consts_late()
                steps.append((0.0, load_x))

            for grp in (2, 0, 1):
                for e in range(NE):
                    def mm(q=q, grp=grp, e=e):
                        if e == 0:
                            live[("acc", q, grp)] = ps.tile(
                                [128, TQ], F32, tag="acc", bufs=2,
                                name=f"acc{q}_{grp}")
                        acc = live[("acc", q, grp)]
                        if grp < 2:
                            lhs = wq_res[:, e, grp * 128:(grp + 1) * 128]
                        else:
                            lhs = wkv_res[:, e, :]
                        _MM_LABELS.append(f"P{q}g{grp}e{e}")
                        nc.tensor.matmul(acc, lhs, live[("xt", q)][:, e, :],
                                         start=(e == 0), stop=(e == NE - 1))
                    steps.append((213.0, mm))

                def stage(q=q, grp=grp, cs=cs, ce=ce):
                    acc = live.pop(("acc", q, grp))
                    if grp < 2:
                        nc.vector.tensor_copy(
                            out=qTa[2 * grp][0:64, cs:ce], in_=acc[0:64, :])
                        nc.vector.tensor_copy(
                            out=qTa[2 * grp + 1][0:64, cs:ce],
                            in_=acc[64:128, :])
                    else:
                        nc.vector.tensor_copy(
                            out=kTa[0:64, cs:ce], in_=acc[0:64, :])
                        stv = sp.tile([64, TQ], BF16, tag="stv",
                                      name=f"stv{q}")
                        nc.vector.tensor_copy(out=stv, in_=acc[64:128, :])
                        for m in range(4):
                            nc.scalar.dma_start_transpose(
                                out=v_aug[:, 4 * q + m, 0:64],
                                in_=stv[:, m * 128:(m + 1) * 128])
                steps.append((0.0, stage))
        return steps

    # ------- scores + AV stream (1024-wide score/exp tiles, q2 blocks) --
    QB = 2 * TQ  # 1024-query score block

    def score_steps_hq(q2, h):
        """Score+exp per (head, k-tile) over a 1024-query block: one exp
        per k-tile (bias is per (h, tk)), 1-2 matmuls (psum-bank split)."""
        steps = []
        for tk in range(8 * q2 + 8):
            qs = max(q2 * QB, tk * 128)
            n = (q2 + 1) * QB - qs

            def step(h=h, q2=q2, tk=tk, qs=qs, n=n):
                if tk == 0:
                    live[("g", h)] = gp.tile(
                        [128, NT128, QB], BF16, tag=f"g{h % 2}",
                        name=f"g{q2}_{h}")
                g = live[("g", h)]
                st = ps.tile([128, QB], F32, tag="st", bufs=2,
                             name=f"st{q2}_{h}_{tk}")
                for c in range(0, n, TQ):
                    ce = min(n, c + TQ)
                    _MM_LABELS.append(f"S{q2}h{h}k{tk}c{c}")
                    nc.tensor.matmul(
                        st[:, c:ce],
                        kTa[:, tk * 128:(tk + 1) * 128],
                        qTa[h][:, qs + c:qs + ce],
                        start=True, stop=True)
                nc.scalar.activation(
                    out=g[:, tk, 0:n], in_=st[:, 0:n], func=EXP,
                    bias=btbl_t[:, h * NT128 + tk:h * NT128 + tk + 1],
                    scale=1.0)
                if tk >= 8 * q2:
                    nc.gpsimd.affine_select(
                        out=g[:, tk, 0:128], in_=g[:, tk, 0:128],
                        compare_op=mybir.AluOpType.is_ge,
                        fill=0.0, base=0,
                        pattern=[[1, 128]], channel_multiplier=-1)
            steps.append((n * 0.4167, step))
        return steps

    def av_steps_hq(q, h):
        """AV + norm + xbar for one 512-query tile q (within block q//2)."""
        steps = []
        q2 = q // 2
        for j in range(4):
            ntk = 4 * q + j + 1

            def mmj(h=h, q=q, q2=q2, j=j, ntk=ntk):
                if j == 0:
                    live[("av", h)] = ps.tile(
                        [128, 4, 65], F32, tag="av", bufs=2,
                        name=f"av{q}_{h}")
                av = live[("av", h)]
                g = live[("g", h)]
                for tk in range(ntk):
                    qs = max(q2 * QB, tk * 128)
                    off = q * TQ + j * 128 - qs
                    _MM_LABELS.append(f"A{q}h{h}j{j}k{tk}")
                    nc.tensor.matmul(
                        av[:, j, :],
                        g[:, tk, off:off + 128],
                        v_aug[:, tk, 0:65],
                        start=(tk == 0), stop=(tk == ntk - 1))
            steps.append((ntk * 65 * 0.4167, mmj))

        def norm(h=h, q=q):
            av = live.pop(("av", h))
            rc = sp.tile([128, 4], F32, tag=f"rc{h}", name=f"rc{q}{h}")
            nc.vector.reciprocal(out=rc, in_=av[:, :, 64])
            if h % 2 == 0:
                live[("hsb", h // 2, q)] = sp.tile(
                    [128, 4, 128], BF16, tag=f"hsb{h // 2}",
                    name=f"hsb{q}{h}", bufs=4)
            hsb = live[("hsb", h // 2, q)]
            rc_b = bass.AP(tensor=rc.tensor, offset=rc.offset,
                           ap=[rc.ap[0], [1, 4], [0, 64]])
            half = (h % 2) * 64
            nc.vector.tensor_mul(
                out=hsb[:, :, half:half + 64], in0=av[:, :, 0:64],
                in1=rc_b)
            if h % 2 == 1:
                # one xbar per head pair:
                # otn[pair][64*(h%2)+d, q*TQ + m*128 + p] = hsb[p, m, 64*(h%2)+d]
                live.pop(("hsb", h // 2, q))
                osl = otn[h // 2][:, q * TQ:(q + 1) * TQ]
                oap = bass.AP(tensor=osl.tensor, offset=osl.offset,
                              ap=[osl.ap[0], [128, 4], [1, 128]])
                nc.sync.dma_start_transpose(out=oap, in_=hsb[:, :, :])
        steps.append((0.0, norm))
        return steps

    def sa_steps(wo, pfill, p3):
        """Per 1024-query block: scores per head, AV per 512-half lagged
        behind; W(old tiles) and remaining proj merged in as PE filler."""
        steps = []
        for q2 in range(2):
            s = [score_steps_hq(q2, h) for h in range(HL)]
            a = [[av_steps_hq(2 * q2 + m2, h) for m2 in range(2)]
                 for h in range(HL)]
            # both AV halves of head h complete before scores(h+2), which
            # reuses h's g slot
            sa = (s[0] + s[1] + a[0][0] + a[0][1]
                  + s[2] + a[1][0] + a[1][1] + s[3] + a[2][0] + a[2][1])
            end = a[3][0] + a[3][1]
            m1, m2_ = [], []
            # wo[q even] must be emitted after a[3][0] (its last xbar)
            if q2 == 0:
                _merge([(sa, 1.0), (pfill, 2.3), (p3, 1.2)], collect=m1)
                _merge([(a[3][1], 1.0), (wo[0], 0.6)], collect=m2_)
            else:
                _merge([(sa, 1.0), (wo[1], 0.75)], collect=m1)
                _merge([(a[3][1], 1.0), (wo[2], 0.9)], collect=m2_)
            m2_ = a[3][0] + m2_
            steps += m1 + m2_
        steps += wo[3]
        return steps

    # ---------------- Wo stream (per source q-tile) ----------------
    def wo_steps_q(qsrc):
        steps = []
        for t in range(4 * qsrc, 4 * qsrc + 4):
            for o in range(4):
                def mmo(t=t, o=o):
                    if o == 0:
                        live[("ob", t)] = obp.tile(
                            [128, E], BF16, tag="ob", name=f"ob{t}")
                    ob = live[("ob", t)]
                    po = ps.tile([128, TQ], F32, tag="acc", bufs=2,
                                 name=f"po{t}_{o}")
                    _MM_LABELS.append(f"W{t}o{o}p0")
                    nc.tensor.matmul(
                        po, otn[0][:, t * 128:(t + 1) * 128],
                        wo_res[0][:, o * TQ:(o + 1) * TQ],
                        start=True, stop=False)
                    _MM_LABELS.append(f"W{t}o{o}p1")
                    nc.tensor.matmul(
                        po, otn[1][:, t * 128:(t + 1) * 128],
                        wo_res[1][:, o * TQ:(o + 1) * TQ],
                        start=False, stop=True)
                    on_act = (o == 3) or (t >= 12 and o == 1)
                    if on_act:
                        nc.scalar.copy(out=ob[:, o * TQ:(o + 1) * TQ],
                                       in_=po)
                    else:
                        nc.vector.tensor_copy(
                            out=ob[:, o * TQ:(o + 1) * TQ], in_=po)
                    if o % 2 == 1:
                        half = (o - 1) * TQ
                        nc.sync.dma_start(
                            out=part[t * 128:(t + 1) * 128,
                                     half:half + 2 * TQ],
                            in_=ob[:, half:half + 2 * TQ])
                    if o == 3:
                        live.pop(("ob", t))
                steps.append((427.0, mmo))
        return steps

    sW = [wo_steps_q(q) for q in range(NTQ)]
    sP1 = proj_steps_q(1)
    nboot1 = 1 + 2 * (NE + 1)       # load_x + grp2 + grp0
    pfill = sP1[nboot1:] + proj_steps_q(2)
    sSA = sa_steps(sW, pfill, proj_steps_q(3))

    # bootstrap: chunked const loads + P(0) + most of P(1)
    load_consts_early()
    for _c, fn in proj_steps_q(0):
        fn()
    for _c, fn in sP1[:nboot1]:
        fn()
    for _c, fn in sSA:
        fn()


def _prepare_in_maps(x, Wq, Wk, Wv, Wo):
    bf = ml_dtypes.bfloat16
    xTn = np.ascontiguousarray(x[0].T).astype(bf)
    wkvTn = np.ascontiguousarray(
        np.concatenate([Wk, Wv], axis=0).T).astype(bf)
    scale = np.float64(D) ** -0.5
    in_maps = []
    for c in range(NCORES):
        sl = slice(c * ES, (c + 1) * ES)
        wqTn = np.ascontiguousarray(
            (Wq[sl, :].astype(np.float64) * scale).T).astype(bf)
        woTn = np.ascontiguousarray(Wo[:, sl].T).astype(bf)
        g = np.arange(c * HL, (c + 1) * HL, dtype=np.float64)
        slopes = np.power(2.0, -8.0 * (g + 1.0) / H)          # [HL]
        i = np.arange(T, dtype=np.float64)
        qaug_n = (-slopes[:, None] * i[None, :]).astype(bf)   # [HL, T]
        p = np.arange(128, dtype=np.float64)
        k = np.arange(NT128, dtype=np.float64)
        btbl_n = (slopes[:, None, None]
                  * (k[None, :, None] * 128 + p[None, None, :]))
        btbl_n = np.ascontiguousarray(
            btbl_n.transpose(2, 0, 1).reshape(128, HL * NT128)
        ).astype(np.float32)
        in_maps.append({
            "xT": xTn, "wqT": wqTn, "wkvT": wkvTn, "woT": woTn,
            "qaug": qaug_n, "ones": np.ones((1, T), dtype=bf),
            "btbl": btbl_n,
        })
    return in_maps


def kernel(x, Wq, Wk, Wv, Wo, attention_mask, _trace=False, _trace_cores=None):
    x = np.asarray(x, dtype=np.float32)
    Wq = np.asarray(Wq, dtype=np.float32)
    Wk = np.asarray(Wk, dtype=np.float32)
    Wv = np.asarray(Wv, dtype=np.float32)
    Wo = np.asarray(Wo, dtype=np.float32)

    if "nc" not in _CACHE:
        _CACHE["nc"] = _build_nc()
    nc = _CACHE["nc"]

    in_maps = _prepare_in_maps(x, Wq, Wk, Wv, Wo)
    kwargs = {}
    if _trace:
        kwargs = {"trace": True, "trace_cores": _trace_cores or [0]}
    res = run_bass_kernel_spmd(nc, in_maps, core_ids=list(range(NCORES)),
                               **kwargs)
    acc = np.zeros((T, E), dtype=np.float32)
    for r in res.results:
        acc += np.asarray(r["part"]).astype(np.float32)
    out = acc[None, :, :]
    if _trace:
        _CACHE["last_result"] = res
    return out


# revision 47
# speedup vs baseline: 1.0044x; 1.0044x over previous
"""MQA attention (32 query heads, 1 KV head, ALiBi, causal) on 8 trn2 cores.

Sharding: tensor-parallel over query heads (4 heads/core). Wq rows and Wo
columns are sharded; x, Wk, Wv are replicated. Each core computes a partial
[T, E] output (its 4 heads through its Wo column-shard) in bf16; the host
sums the 8 partials in fp32.

All matmuls run in bf16 (1 cycle/row on PE regardless of width). Per core:

  proj (PE):   qT_h = (Wq_h D^-.5) @ x^T [64,T]/head, kvT = Wkv @ x^T [128,T]
  scores (PE): ST[k,q] = kTa[:,kblk]^T qTa[:,qcols]  (aug row: ones x -s_h i)
  exp (ACT):   g = exp(ST + s_h(128 tk + p))  bias per (head, k-tile)
  mask (Pool): affine_select zeroes j>i on diagonal 128-blocks
  AV (PE):     av[128q, 65] += g_blk[128k,128q]^T @ v_aug[128k,65]
               (col 64 of v_aug is ones -> softmax denominator)
  norm (DVE):  hsb[p, m, d] = av[p,m,d] * recip(av[p,m,64])
  transpose:   DMA-xbar hsb -> otn[d, qcols] (headout^T, per (h,q))
  Wo (PE):     part[t,:] = otn0[:,tblk]^T wo0 + otn1[:,tblk]^T wo1

The -s_h*i aug row cancels exactly in softmax normalization (per-query
constant), so bf16 rounding of it is harmless. Emission interleaves three
streams (proj / scores+AV / Wo) with a proportional merge so the PE stays
fed while ACT drains the exps.
"""

import numpy as np
import ml_dtypes

import concourse.bacc as bacc
import concourse.bass as bass
import concourse.mybir as mybir
import concourse.tile as tile
from concourse.bass_utils import run_bass_kernel_spmd

T = 2048
E = 2048
H = 32
D = 64
NCORES = 8
HL = H // NCORES   # 4 heads per core
ES = HL * D        # 256 per-core E shard
TQ = 512           # query tile
NTQ = T // TQ      # 4
NE = E // 128      # 16 contraction chunks
NT128 = T // 128   # 16

F32 = mybir.dt.float32
BF16 = mybir.dt.bfloat16
EXP = mybir.ActivationFunctionType.Exp

_CACHE = {}
_MM_LABELS = []

# merge weights: higher -> stream finishes earlier in the emission
W_PROJ = 2.6
W_SA = 1.0


def _merge(streams, collect=None):
    """streams: list of (steps, weight); steps = list of (cost, fn).
    Emits every fn once (or appends to `collect`), proportionally by
    weighted cumulative cost."""
    totals = [max(sum(c for c, _ in s), 1e-9) * w for s, w in streams]
    done = [0.0] * len(streams)
    idx = [0] * len(streams)
    while True:
        best, bestv = -1, None
        for i, (s, _w) in enumerate(streams):
            if idx[i] >= len(s):
                continue
            v = done[i] / totals[i]
            if best < 0 or v < bestv:
                best, bestv = i, v
        if best < 0:
            return
        step = streams[best][0][idx[best]]
        idx[best] += 1
        done[best] += step[0]
        if collect is not None:
            collect.append(step)
        else:
            step[1]()


def _build_nc():
    nc = bacc.Bacc("TRN2")
    xT = nc.dram_tensor("xT", [E, T], BF16, kind="ExternalInput")
    wqT = nc.dram_tensor("wqT", [E, ES], BF16, kind="ExternalInput")
    wkvT = nc.dram_tensor("wkvT", [E, 2 * D], BF16, kind="ExternalInput")
    woT = nc.dram_tensor("woT", [ES, E], BF16, kind="ExternalInput")
    qaug = nc.dram_tensor("qaug", [HL, T], BF16, kind="ExternalInput")
    ones = nc.dram_tensor("ones", [1, T], BF16, kind="ExternalInput")
    btbl = nc.dram_tensor("btbl", [128, HL * NT128], F32, kind="ExternalInput")
    part = nc.dram_tensor("part", [T, E], BF16, kind="ExternalOutput")

    from contextlib import ExitStack
    with tile.TileContext(nc) as tc, ExitStack() as ctx:
        _body(nc, tc, ctx, xT, wqT, wkvT, woT, qaug, ones, btbl, part)
    nc.finalize()
    return nc


def _body(nc, tc, ctx, xT, wqT, wkvT, woT, qaug, ones, btbl, part):
    const = ctx.enter_context(tc.tile_pool(name="const", bufs=1))
    xtp = ctx.enter_context(tc.tile_pool(name="xt", bufs=2))
    gp = ctx.enter_context(tc.tile_pool(name="g", bufs=1))
    sp = ctx.enter_context(tc.tile_pool(name="stage", bufs=2))
    obp = ctx.enter_context(tc.tile_pool(name="ob", bufs=2))
    ps = ctx.enter_context(tc.tile_pool(name="ps", bufs=1, space="PSUM"))

    # ---------------- resident constants (DMAs emitted in bootstrap) ----
    wq_res = const.tile([128, NE, ES], BF16)
    wkv_res = const.tile([128, NE, 2 * D], BF16)
    wo_res = [const.tile([128, E], BF16, tag=f"wo{p2}", name=f"wo{p2}")
              for p2 in range(2)]
    qTa = [const.tile([65, T], BF16, tag=f"qTa{h}", name=f"qTa{h}")
           for h in range(HL)]
    kTa = const.tile([65, T], BF16)
    # v_aug: [128 keys, k-tile, 128 slot]; col 64 = ones (denominator row)
    v_aug = const.tile([128, NT128, 128], BF16)
    btbl_t = const.tile([128, HL * NT128], F32)
    otn = [const.tile([128, T], BF16, tag=f"otn{p2}", name=f"otn{p2}")
           for p2 in range(2)]

    # live tiles, stashed at emission time by the creating step
    live = {}

    def load_consts_early():
        # kv weights first (P(0) starts with the kv group), then x chunks
        # interleaved with wq so each proj group starts as data lands
        xt0 = xtp.tile([128, NE, TQ], BF16, tag="xt", name="xt0")
        live[("xt", 0)] = xt0
        for e8 in range(2):
            sl = slice(e8 * 1024, (e8 + 1) * 1024)
            nc.sync.dma_start(
                out=wkv_res[:, 8 * e8:8 * e8 + 8, :],
                in_=wkvT[sl, :].rearrange("(e p) m -> p e m", p=128))
        for e4 in range(NE // 4):
            sl = slice(e4 * 512, (e4 + 1) * 512)
            nc.sync.dma_start(
                out=xt0[:, 4 * e4:4 * e4 + 4, :],
                in_=xT[sl, 0:TQ].rearrange("(e p) t -> p e t", p=128))
            nc.sync.dma_start(
                out=wq_res[:, 4 * e4:4 * e4 + 4, :],
                in_=wqT[sl, :].rearrange("(e p) m -> p e m", p=128))

    def load_consts_mid():
        for h in range(HL):
            nc.sync.dma_start(out=qTa[h][64:65, :], in_=qaug[h:h + 1, :])
        nc.sync.dma_start(out=btbl_t, in_=btbl[:, :])
        nc.gpsimd.memset(kTa[64:65, :], 1.0)
        nc.gpsimd.memset(v_aug[:, :, 64:65], 1.0)

    def load_consts_late():
        for p2 in range(2):
            nc.sync.dma_start(out=wo_res[p2],
                              in_=woT[p2 * 128:(p2 + 1) * 128, :])

    # ---------------- proj stream (kv group first) ----------------
    def proj_steps_q(q):
        steps = []
        if True:
            cs, ce = q * TQ, (q + 1) * TQ

            if q > 0:
                def load_x(q=q, cs=cs, ce=ce):
                    xt = xtp.tile([128, NE, TQ], BF16, tag="xt",
                                  name=f"xt{q}")
                    live[("xt", q)] = xt
                    for e8 in range(2):
                        sl = slice(e8 * 1024, (e8 + 1) * 1024)
                        nc.sync.dma_start(
                            out=xt[:, 8 * e8:8 * e8 + 8, :],
                            in_=xT[sl, cs:ce].rearrange(
                                "(e p) t -> p e t", p=128))
                    if q == 1:
                        load_consts_mid()
                    if q == 2:
                        load_consts_late()
                steps.append((0.0, load_x))

            for grp in (2, 0, 1):
                for e in range(NE):
                    def mm(q=q, grp=grp, e=e):
                        if e == 0:
                            live[("acc", q, grp)] = ps.tile(
                                [128, TQ], F32, tag="acc", bufs=2,
                                name=f"acc{q}_{grp}")
                        acc = live[("acc", q, grp)]
                        if grp < 2:
                            lhs = wq_res[:, e, grp * 128:(grp + 1) * 128]
                        else:
                            lhs = wkv_res[:, e, :]
                        _MM_LABELS.append(f"P{q}g{grp}e{e}")
                        nc.tensor.matmul(acc, lhs, live[("xt", q)][:, e, :],
                                         start=(e == 0), stop=(e == NE - 1))
                    steps.append((213.0, mm))

                def stage(q=q, grp=grp, cs=cs, ce=ce):
                    acc = live.pop(("acc", q, grp))
                    if grp < 2:
                        nc.vector.tensor_copy(
                            out=qTa[2 * grp][0:64, cs:ce], in_=acc[0:64, :])
                        nc.vector.tensor_copy(
                            out=qTa[2 * grp + 1][0:64, cs:ce],
                            in_=acc[64:128, :])
                    else:
                        nc.vector.tensor_copy(
                            out=kTa[0:64, cs:ce], in_=acc[0:64, :])
                        stv = sp.tile([64, TQ], BF16, tag="stv",
                                      name=f"stv{q}")
                        nc.vector.tensor_copy(out=stv, in_=acc[64:128, :])
                        for m in range(4):
                            nc.scalar.dma_start_transpose(
                                out=v_aug[:, 4 * q + m, 0:64],
                                in_=stv[:, m * 128:(m + 1) * 128])
                steps.append((0.0, stage))
        return steps

    # ------- scores + AV stream (1024-wide score/exp tiles, q2 blocks) --
    QB = 2 * TQ  # 1024-query score block

    def score_steps_hq(q2, h):
        """Score+exp per (head, k-tile) over a 1024-query block: one exp
        per k-tile (bias is per (h, tk)), 1-2 matmuls (psum-bank split)."""
        steps = []
        for tk in range(8 * q2 + 8):
            qs = max(q2 * QB, tk * 128)
            n = (q2 + 1) * QB - qs

            def step(h=h, q2=q2, tk=tk, qs=qs, n=n):
                if tk == 0:
                    live[("g", h)] = gp.tile(
                        [128, NT128, QB], BF16, tag=f"g{h % 2}",
                        name=f"g{q2}_{h}")
                g = live[("g", h)]
                st = ps.tile([128, QB], F32, tag="st", bufs=2,
                             name=f"st{q2}_{h}_{tk}")
                for c in range(0, n, TQ):
                    ce = min(n, c + TQ)
                    _MM_LABELS.append(f"S{q2}h{h}k{tk}c{c}")
                    nc.tensor.matmul(
                        st[:, c:ce],
                        kTa[:, tk * 128:(tk + 1) * 128],
                        qTa[h][:, qs + c:qs + ce],
                        start=True, stop=True)
                nc.scalar.activation(
                    out=g[:, tk, 0:n], in_=st[:, 0:n], func=EXP,
                    bias=btbl_t[:, h * NT128 + tk:h * NT128 + tk + 1],
                    scale=1.0)
                if tk >= 8 * q2:
                    nc.gpsimd.affine_select(
                        out=g[:, tk, 0:128], in_=g[:, tk, 0:128],
                        compare_op=mybir.AluOpType.is_ge,
                        fill=0.0, base=0,
                        pattern=[[1, 128]], channel_multiplier=-1)
            steps.append((n * 0.4167, step))
        return steps

    def av_steps_hq(q, h):
        """AV + norm + xbar for one 512-query tile q (within block q//2)."""
        steps = []
        q2 = q // 2
        for j in range(4):
            ntk = 4 * q + j + 1

            def mmj(h=h, q=q, q2=q2, j=j, ntk=ntk):
                if j == 0:
                    live[("av", h)] = ps.tile(
                        [128, 4, 65], F32, tag="av", bufs=2,
                        name=f"av{q}_{h}")
                av = live[("av", h)]
                g = live[("g", h)]
                for tk in range(ntk):
                    qs = max(q2 * QB, tk * 128)
                    off = q * TQ + j * 128 - qs
                    _MM_LABELS.append(f"A{q}h{h}j{j}k{tk}")
                    nc.tensor.matmul(
                        av[:, j, :],
                        g[:, tk, off:off + 128],
                        v_aug[:, tk, 0:65],
                        start=(tk == 0), stop=(tk == ntk - 1))
            steps.append((ntk * 65 * 0.4167, mmj))

        def norm(h=h, q=q):
            av = live.pop(("av", h))
            rc = sp.tile([128, 4], F32, tag=f"rc{h}", name=f"rc{q}{h}")
            nc.vector.reciprocal(out=rc, in_=av[:, :, 64])
            if h % 2 == 0:
                live[("hsb", h // 2, q)] = sp.tile(
                    [128, 4, 128], BF16, tag=f"hsb{h // 2}",
                    name=f"hsb{q}{h}", bufs=4)
            hsb = live[("hsb", h // 2, q)]
            rc_b = bass.AP(tensor=rc.tensor, offset=rc.offset,
                           ap=[rc.ap[0], [1, 4], [0, 64]])
            half = (h % 2) * 64
            nc.vector.tensor_mul(
                out=hsb[:, :, half:half + 64], in0=av[:, :, 0:64],
                in1=rc_b)
            if h % 2 == 1:
                # one xbar per head pair:
                # otn[pair][64*(h%2)+d, q*TQ + m*128 + p] = hsb[p, m, 64*(h%2)+d]
                live.pop(("hsb", h // 2, q))
                osl = otn[h // 2][:, q * TQ:(q + 1) * TQ]
                oap = bass.AP(tensor=osl.tensor, offset=osl.offset,
                              ap=[osl.ap[0], [128, 4], [1, 128]])
                nc.sync.dma_start_transpose(out=oap, in_=hsb[:, :, :])
        steps.append((0.0, norm))
        return steps

    def sa_steps(wo, pfill, p3):
        """Per 1024-query block: scores per head, AV per 512-half lagged
        behind; W(old tiles) and remaining proj merged in as PE filler."""
        steps = []
        for q2 in range(2):
            s = [score_steps_hq(q2, h) for h in range(HL)]
            a = [[av_steps_hq(2 * q2 + m2, h) for m2 in range(2)]
                 for h in range(HL)]
            # both AV halves of head h complete before scores(h+2), which
            # reuses h's g slot
            sa = (s[0] + s[1] + a[0][0] + a[0][1]
                  + s[2] + a[1][0] + a[1][1] + s[3] + a[2][0] + a[2][1])
            end = a[3][0] + a[3][1]
            m1, m2_ = [], []
            # wo[q even] must be emitted after a[3][0] (its last xbar)
            if q2 == 0:
                _merge([(sa, 1.0), (pfill, 1.9), (p3, 1.0)], collect=m1)
                _merge([(a[3][1], 1.0), (wo[0], 0.9)], collect=m2_)
            else:
                _merge([(sa, 1.0), (wo[1], 0.75)], collect=m1)
                _merge([(a[3][1], 1.0), (wo[2], 0.9)], collect=m2_)
            m2_ = a[3][0] + m2_
            steps += m1 + m2_
        steps += wo[3]
        return steps

    # ---------------- Wo stream (per source q-tile) ----------------
    def wo_steps_q(qsrc):
        steps = []
        for t in range(4 * qsrc, 4 * qsrc + 4):
            for o in range(4):
                def mmo(t=t, o=o):
                    if o == 0:
                        live[("ob", t)] = obp.tile(
                            [128, E], BF16, tag="ob", name=f"ob{t}")
                    ob = live[("ob", t)]
                    po = ps.tile([128, TQ], F32, tag="acc", bufs=2,
                                 name=f"po{t}_{o}")
                    _MM_LABELS.append(f"W{t}o{o}p0")
                    nc.tensor.matmul(
                        po, otn[0][:, t * 128:(t + 1) * 128],
                        wo_res[0][:, o * TQ:(o + 1) * TQ],
                        start=True, stop=False)
                    _MM_LABELS.append(f"W{t}o{o}p1")
                    nc.tensor.matmul(
                        po, otn[1][:, t * 128:(t + 1) * 128],
                        wo_res[1][:, o * TQ:(o + 1) * TQ],
                        start=False, stop=True)
                    on_act = (o == 3) or (t >= 12 and o == 1)
                    if on_act:
                        nc.scalar.copy(out=ob[:, o * TQ:(o + 1) * TQ],
                                       in_=po)
                    else:
                        nc.vector.tensor_copy(
                            out=ob[:, o * TQ:(o + 1) * TQ], in_=po)
                    if o % 2 == 1:
                        half = (o - 1) * TQ
                        nc.sync.dma_start(
                            out=part[t * 128:(t + 1) * 128,
                                     half:half + 2 * TQ],
                            in_=ob[:, half:half + 2 * TQ])
                    if o == 3:
                        live.pop(("ob", t))
                steps.append((427.0, mmo))
        return steps

    sW = [wo_steps_q(q) for q in range(NTQ)]
    sP1 = proj_steps_q(1)
    nboot1 = 1 + 2 * (NE + 1)       # load_x + grp2 + grp0
    pfill = sP1[nboot1:] + proj_steps_q(2)
    sSA = sa_steps(sW, pfill, proj_steps_q(3))

    # bootstrap: chunked const loads + P(0) + most of P(1)
    load_consts_early()
    for _c, fn in proj_steps_q(0):
        fn()
    for _c, fn in sP1[:nboot1]:
        fn()
    for _c, fn in sSA:
        fn()


def _prepare_in_maps(x, Wq, Wk, Wv, Wo):
    bf = ml_dtypes.bfloat16
    xTn = np.ascontiguousarray(x[0].T).astype(bf)
    wkvTn = np.ascontiguousarray(
        np.concatenate([Wk, Wv], axis=0).T).astype(bf)
    scale = np.float64(D) ** -0.5
    in_maps = []
    for c in range(NCORES):
        sl = slice(c * ES, (c + 1) * ES)
        wqTn = np.ascontiguousarray(
            (Wq[sl, :].astype(np.float64) * scale).T).astype(bf)
        woTn = np.ascontiguousarray(Wo[:, sl].T).astype(bf)
        g = np.arange(c * HL, (c + 1) * HL, dtype=np.float64)
        slopes = np.power(2.0, -8.0 * (g + 1.0) / H)          # [HL]
        i = np.arange(T, dtype=np.float64)
        qaug_n = (-slopes[:, None] * i[None, :]).astype(bf)   # [HL, T]
        p = np.arange(128, dtype=np.float64)
        k = np.arange(NT128, dtype=np.float64)
        btbl_n = (slopes[:, None, None]
                  * (k[None, :, None] * 128 + p[None, None, :]))
        btbl_n = np.ascontiguousarray(
            btbl_n.transpose(2, 0, 1).reshape(128, HL * NT128)
        ).astype(np.float32)
        in_maps.append({
            "xT": xTn, "wqT": wqTn, "wkvT": wkvTn, "woT": woTn,
            "qaug": qaug_n, "ones": np.ones((1, T), dtype=bf),
            "btbl": btbl_n,
        })
    return in_maps


def kernel(x, Wq, Wk, Wv, Wo, attention_mask, _trace=False, _trace_cores=None):
    x = np.asarray(x, dtype=np.float32)
    Wq = np.asarray(Wq, dtype=np.float32)
    Wk = np.asarray(Wk, dtype=np.float32)
    Wv = np.asarray(Wv, dtype=np.float32)
    Wo = np.asarray(Wo, dtype=np.float32)

    if "nc" not in _CACHE:
        _CACHE["nc"] = _build_nc()
    nc = _CACHE["nc"]

    in_maps = _prepare_in_maps(x, Wq, Wk, Wv, Wo)
    kwargs = {}
    if _trace:
        kwargs = {"trace": True, "trace_cores": _trace_cores or [0]}
    res = run_bass_kernel_spmd(nc, in_maps, core_ids=list(range(NCORES)),
                               **kwargs)
    acc = np.zeros((T, E), dtype=np.float32)
    for r in res.results:
        acc += np.asarray(r["part"]).astype(np.float32)
    out = acc[None, :, :]
    if _trace:
        _CACHE["last_result"] = res
    return out
